# revision 26
# baseline (speedup 1.0000x reference)
"""Trainium2 Bass kernel for a StyleGAN-style modulated conv2d.

Reference math (see problem statement):
    w  = kernel * he_std                       # equalized-lr
    s  = style @ w_mod + b_mod + 1             # [B, cin]
    s  = s / max|s|                            # global max-abs over [B, cin]
    w  = w * s[0][None, None, :, None]         # style[0] only -> one shared weight
    d  = rsqrt(sum(w^2, (0,1,2)) + 1e-8)
    w  = w * d
    y  = conv2d_same(x, w) + noise*(ns/2) + bias
    y  = lrelu(y, 0.2) * sqrt(2)

Because only style[0] modulates, the effective 3x3x128x128 weight is identical
for every batch element, so the device work is a plain 3x3 conv. The tiny
modulation math (a 512x128 matvec + norms, ~1e-6 of total FLOPs) is folded on
the host while sharding; the conv + activation run on 8 NeuronCores,
data-parallel over batch (1 image per core).

Device strategy per core (v3):
  - x is pre-padded/transposed on the host to [cin=128, 258, 258] bf16 (zero
    SAME-padding baked in), so every DMA is a plain linear per-partition copy.
  - 3x3 conv = 9 accumulating matmuls per PSUM group: lhsT = w[cin,cout] per
    tap, rhs = shifted x rows ([2 rows x 256 cols] = 512 spatial AP), PSUM
    [cout=128, 512] fp32. Mid-stream the MM gap is the N=512 streaming floor
    (~216 ns), so the remaining time lives in the head and tail:
  - sqrt(2) is folded into the weights, so the epilogue is a single ScalarE
    op per group: y = Prelu(psum [+ sqrt2*bias], alpha=0.2), written directly
    as bf16 (host upcasts to fp32).
  - The image's first and last rows run as 1-row groups whose out-of-image
    tap rows (zero padding) are skipped: 6 matmuls instead of 9.
  - Head: slab 0's x DMA is split into 4 chunks (6/10/10/14 rows), and the
    first chunk is further split by partition halves across the two HWDGE
    queues (Sync + Scalar) so the first matmuls only wait ~3us of parallel
    DMA after queue spin-up (the GpSimd SWDGE queue spins up too slowly for
    head-critical data); ~42 dummy warm-up matmuls on a memset tile keep
    TensorE busy through the DMA spin-up so HAM is at K=8/8 (2.4 GHz) when
    the real matmuls start, with no idle gap long enough to let the HAM MID
    window re-throttle the PE.
  - Tail: output is DMA'd per 2-row group (128KB bf16) on the GpSimd DMA
    queue (decoupled from input prefetch); the last two groups go out on the
    by-then-idle Sync queue so the GpSimd queue drain doesn't serialize
    behind the final transfer.
"""

import math
from contextlib import ExitStack

import ml_dtypes
import numpy as np

import concourse.bacc as bacc
import concourse.bass as bass
import concourse.mybir as mybir
import concourse.tile as tile
from concourse.bass_utils import run_bass_kernel_spmd

B, H, W, CIN, COUT, KK, SDIM = 8, 256, 256, 128, 128, 3, 512
HP, WP = H + 2, W + 2  # zero-padded spatial dims (SAME padding for 3x3)
N_CORES = 8
ROWS_PER_SLAB = 32          # output rows per input slab
SLABS = H // ROWS_PER_SLAB  # 8
GROUP_ROWS = 2              # output rows per PSUM group (2*256 = 512 = 1 bank)
GROUPS_PER_SLAB = ROWS_PER_SLAB // GROUP_ROWS  # 16
N_WARMUP_MM = 54            # dummy matmuls covering DMA spin-up; sized to end
                            # at ~p90 of head-data arrival: ending early risks
                            # a HAM re-throttle (~1.8us), overshoot costs only
                            # the difference at ~56ns granularity
N_SYNCQ_TAIL_GROUPS = 2     # last groups whose output DMA rides the Sync queue
N_TAIL_SINGLE_ROWS = 2      # last output rows computed as 1-row groups (short tail)

# slab-0 chunks: (first input row, rows, first group, groups). Group g reads
# input rows 2g..2g+3; chunks overlap by 2 rows so every group is whole.
S0_CHUNKS = [(0, 6, 0, 2), (4, 10, 2, 4), (12, 10, 6, 4), (20, 14, 10, 6)]

BF16 = mybir.dt.bfloat16
F32 = mybir.dt.float32
SQRT2 = float(np.sqrt(np.float32(2.0)))


def _effective_weight(style, kernel, w_mod, b_mod):
    """Exactly the reference weight math, in fp32 numpy."""
    style = np.asarray(style, np.float32)
    kernel = np.asarray(kernel, np.float32)
    w_mod = np.asarray(w_mod, np.float32)
    b_mod = np.asarray(b_mod, np.float32)

    he_std = np.float32(1.0) / np.sqrt(np.float32(KK * KK * CIN))
    w = kernel * he_std
    s = (style @ w_mod + b_mod + np.float32(1.0)).astype(np.float32)
    s = s * (np.float32(1.0) / np.max(np.abs(s)))
    w = w * s[0][None, None, :, None]
    d = np.float32(1.0) / np.sqrt(
        np.sum(np.square(w), axis=(0, 1, 2), dtype=np.float32) + np.float32(1e-8)
    )
    w = w * d[None, None, None, :]
    return w.astype(np.float32)  # [3, 3, cin, cout]


def _build_program(with_noise: bool, with_bias: bool):
    # Bacc (not raw Bass): its compile() splits multi-sem sync waits into
    # event semaphores — TRN2 allows at most one wait per instruction.
    nc = bacc.Bacc(trn_type="TRN2")
    x = nc.declare_dram_parameter("x", [CIN, HP * WP], BF16, isOutput=False)
    w = nc.declare_dram_parameter("w", [CIN, 9 * COUT], BF16, isOutput=False)
    if with_bias:
        ab = nc.declare_dram_parameter("ab", [COUT, 1], F32, isOutput=False)
    if with_noise:
        nz = nc.declare_dram_parameter("nz", [1, H * W], BF16, isOutput=False)
        ones = nc.declare_dram_parameter("ones", [1, COUT], BF16, isOutput=False)
    y = nc.declare_dram_parameter("y", [COUT, H * W], BF16, isOutput=True)

    slab_rows_in = ROWS_PER_SLAB + 2  # input halo rows per slab

    with ExitStack() as ctx:
        tc = ctx.enter_context(tile.TileContext(nc))
        consts = ctx.enter_context(tc.tile_pool(name="consts", bufs=1))
        s0pool = ctx.enter_context(tc.tile_pool(name="s0", bufs=1))
        xpool = ctx.enter_context(tc.tile_pool(name="x", bufs=3))
        # Output staging: enough buffers to ride out the head transient while
        # the output DMA queue spins up.
        opool = ctx.enter_context(tc.tile_pool(name="out", bufs=10))
        pspool = ctx.enter_context(tc.tile_pool(name="ps", bufs=7, space="PSUM"))
        wupool = ctx.enter_context(tc.tile_pool(name="wups", bufs=1, space="PSUM"))
        if with_noise:
            nzpool = ctx.enter_context(tc.tile_pool(name="nz", bufs=2))

        # PE warm-up: dummy matmuls on a memset tile into a scratch PSUM bank
        # (never read). They run while the DMA queues spin up and the first
        # x chunk streams in, flipping HAM to K=8/8 before the real matmuls.
        # (Tile requires the dummy to be written, so a cheap GpSimd memset.)
        dummy = consts.tile([CIN, COUT], BF16)
        nc.gpsimd.memset(dummy[:], 0.0)
        wps = wupool.tile([COUT, COUT], F32)
        for _ in range(N_WARMUP_MM):
            nc.tensor.matmul(wps[:], dummy[:], dummy[:], start=True, stop=True)

        # Weights (and bias) go first on the Sync HWDGE queue: the GpSimd
        # queue is SWDGE with a multi-us spin-up, so weights parked there
        # arrive after the first x chunk and stall the first matmuls.
        wt = consts.tile([CIN, 9 * COUT], BF16)
        nc.sync.dma_start(wt[:], w[:])
        if with_bias:
            abt = consts.tile([COUT, 1], F32)
            nc.sync.dma_start(abt[:], ab[:])
        if with_noise:
            onest = consts.tile([1, COUT], BF16)
            nc.sync.dma_start(onest[:], ones[:])

        # Slab 0 arrives in small chunks so group 0 starts ~3us earlier. The
        # first chunk is further split by partition halves across the two
        # HWDGE queues (Sync + Scalar) so the head-critical bytes land in
        # parallel with the weight DMA.
        s0_views = []  # (first_group, n_groups, view, first_input_row)
        for ci, (row0, nrows, g0, ngroups) in enumerate(S0_CHUNKS):
            ck = s0pool.tile([CIN, nrows * WP], BF16, name=f"s0c{row0}")
            src = x[:, row0 * WP : (row0 + nrows) * WP]
            if ci == 0:
                # 48/80 split: the Sync queue also carries the 0.3MB weight
                # DMA, the Scalar queue is a bit slower — this equalizes when
                # the two halves (and the weights) finish.
                nc.sync.dma_start(ck[0:48, :], src[0:48, :])
                nc.scalar.dma_start(ck[48:128, :], src[48:128, :])
            else:
                nc.sync.dma_start(ck[:], src)
            s0_views.append(
                (g0, ngroups, ck[:].rearrange("p (r c) -> p r c", c=WP), row0)
            )

        def slab0_view(g):
            for g0, ngroups, view, row0 in s0_views:
                if g0 <= g < g0 + ngroups:
                    return view, 2 * g - row0
            raise AssertionError(g)

        for slab in range(SLABS):
            r0 = slab * ROWS_PER_SLAB  # first output row of the slab
            if slab > 0:
                xt = xpool.tile([CIN, slab_rows_in * WP], BF16)
                nc.sync.dma_start(
                    xt[:], x[:, r0 * WP : (r0 + slab_rows_in) * WP]
                )
                xv = xt[:].rearrange("p (r c) -> p r c", c=WP)
            if with_noise:
                nzt = nzpool.tile([1, ROWS_PER_SLAB * W], BF16)
                nc.sync.dma_start(nzt[:], nz[:, r0 * W : (r0 + ROWS_PER_SLAB) * W])

            # (row-in-slab, n-rows, skip_dh) work items. The image's first and
            # last rows run as 1-row groups: their out-of-image tap rows only
            # multiply the zero padding, so those 3 matmuls are skipped
            # (skip_dh); the last rows' short groups also halve the final
            # ACT + output DMA after the last matmul.
            if slab == 0:
                items = [(0, 1, 0, 0, W), (1, 1, None, 0, W)]
                items += [
                    (2 + g * GROUP_ROWS, GROUP_ROWS, None, 0, W)
                    for g in range((ROWS_PER_SLAB - 2) // GROUP_ROWS)
                ]
            elif slab == SLABS - 1 and N_TAIL_SINGLE_ROWS:
                items = [
                    (g * GROUP_ROWS, GROUP_ROWS, None, 0, W)
                    for g in range(GROUPS_PER_SLAB - N_TAIL_SINGLE_ROWS // GROUP_ROWS)
                ]
                items += [
                    (ROWS_PER_SLAB - N_TAIL_SINGLE_ROWS + k, 1, None, 0, W)
                    for k in range(N_TAIL_SINGLE_ROWS - 1)
                ]
                # image row 255 (padded below -> skip dh=2) as two half-width
                # items, so the post-last-matmul ACT + DMA chain is N=128
                items += [
                    (ROWS_PER_SLAB - 1, 1, 2, 0, W // 2),
                    (ROWS_PER_SLAB - 1, 1, 2, W // 2, W // 2),
                ]
            else:
                items = [
                    (g * GROUP_ROWS, GROUP_ROWS, None, 0, W)
                    for g in range(GROUPS_PER_SLAB)
                ]

            for rr, nrows, skip_dh, c0, ncols in items:
                if slab == 0:
                    gv, grr = slab0_view(rr // GROUP_ROWS)
                    grr += rr % GROUP_ROWS
                else:
                    gv, grr = xv, rr
                ps = pspool.tile([COUT, nrows * ncols], F32)
                taps = [t for t in range(9) if divmod(t, 3)[0] != skip_dh]
                for i, t in enumerate(taps):
                    dh, dw = divmod(t, 3)
                    rhs = gv[:, grr + dh : grr + dh + nrows, dw + c0 : dw + c0 + ncols]
                    nc.tensor.matmul(
                        ps[:],
                        wt[:, t * COUT : (t + 1) * COUT],
                        rhs,
                        start=(i == 0),
                        stop=(i == len(taps) - 1 and not with_noise),
                    )
                if with_noise:
                    nc.tensor.matmul(
                        ps[:],
                        onest[:],
                        nzt[:, rr * W + c0 : rr * W + c0 + nrows * ncols],
                        start=False,
                        stop=True,
                    )
                ot = opool.tile([COUT, nrows * ncols], BF16)
                # weights carry sqrt2: y = prelu(ps + sqrt2*b, 0.2)
                nc.scalar.activation(
                    ot[:],
                    ps[:],
                    mybir.ActivationFunctionType.Prelu,
                    bias=abt[:, 0:1] if with_bias else 0.0,
                    scale=1.0,
                    alpha=0.2,
                )
                row = r0 + rr
                # Outputs ride the GpSimd DMA queue (decoupled from the input
                # prefetch FIFO); the final groups go via the by-then-idle
                # Sync queue so the GpSimd drain doesn't wait on them, and the
                # very last one is issued by the Scalar engine itself — its
                # ACT just produced the data, so no cross-engine sem hop sits
                # on the final critical chain.
                last = (
                    slab == SLABS - 1
                    and rr + nrows > ROWS_PER_SLAB - N_SYNCQ_TAIL_GROUPS * GROUP_ROWS
                )
                final = (
                    slab == SLABS - 1
                    and rr + nrows == ROWS_PER_SLAB
                    and c0 + ncols == W
                )
                eng = nc.scalar if final else (nc.sync if last else nc.gpsimd)
                eng.dma_start(
                    y[:, row * W + c0 : row * W + c0 + nrows * ncols], ot[:]
                )
    nc.finalize()  # Bacc.compile(): reg alloc + split multi-sem waits (TRN2)
    return nc


def _run(inputs, trace=False, **spmd_kwargs):
    x = np.asarray(inputs["x"])
    noise_strength = float(np.asarray(inputs["noise_strength"]).reshape(-1)[0])
    bias = np.asarray(inputs["bias"], np.float32)

    w_eff = _effective_weight(
        inputs["style"], inputs["kernel"], inputs["w_mod"], inputs["b_mod"]
    )
    wscale = np.float32(SQRT2)
    # [3,3,cin,cout] -> [cin, tap*cout], tap-major free dim
    w_dev = np.ascontiguousarray(
        (w_eff * wscale).transpose(2, 0, 1, 3).reshape(CIN, 9 * COUT)
    ).astype(ml_dtypes.bfloat16)

    # Pad + NHWC->NCHW per image, cast bf16. Zero borders bake in SAME padding.
    x_pad = np.zeros((B, CIN, HP, WP), dtype=ml_dtypes.bfloat16)
    x_pad[:, :, 1 : H + 1, 1 : W + 1] = x.transpose(0, 3, 1, 2).astype(
        ml_dtypes.bfloat16
    )

    with_bias = bool(np.any(bias != 0.0))
    with_noise = noise_strength != 0.0
    ab = (bias * wscale).reshape(COUT, 1).astype(np.float32)

    in_maps = []
    for b in range(B):
        m = {
            "x": np.ascontiguousarray(x_pad[b].reshape(CIN, HP * WP)),
            "w": w_dev,
        }
        if with_bias:
            m["ab"] = ab
        if with_noise:
            nzb = np.asarray(inputs["noise"], np.float32)[b, :, :, 0] * np.float32(
                wscale * noise_strength / 2.0
            )
            m["nz"] = nzb.reshape(1, H * W).astype(ml_dtypes.bfloat16)
            m["ones"] = np.ones((1, COUT), dtype=ml_dtypes.bfloat16)
        in_maps.append(m)

    nc = _build_program(with_noise, with_bias)
    res = run_bass_kernel_spmd(
        nc, in_maps, list(range(N_CORES)), trace=trace, **spmd_kwargs
    )

    out = np.empty((B, H, W, COUT), dtype=np.float32)
    for b in range(B):
        out[b] = (
            res.results[b]["y"]
            .astype(np.float32)
            .reshape(COUT, H, W)
            .transpose(1, 2, 0)
        )
    return out, res


def kernel(**inputs):
    out, _ = _run(inputs)
    return out


# revision 29
# speedup vs baseline: 1.0029x; 1.0029x over previous
"""Trainium2 Bass kernel for a StyleGAN-style modulated conv2d.

Reference math (see problem statement):
    w  = kernel * he_std                       # equalized-lr
    s  = style @ w_mod + b_mod + 1             # [B, cin]
    s  = s / max|s|                            # global max-abs over [B, cin]
    w  = w * s[0][None, None, :, None]         # style[0] only -> one shared weight
    d  = rsqrt(sum(w^2, (0,1,2)) + 1e-8)
    w  = w * d
    y  = conv2d_same(x, w) + noise*(ns/2) + bias
    y  = lrelu(y, 0.2) * sqrt(2)

Because only style[0] modulates, the effective 3x3x128x128 weight is identical
for every batch element, so the device work is a plain 3x3 conv. The tiny
modulation math (a 512x128 matvec + norms, ~1e-6 of total FLOPs) is folded on
the host while sharding; the conv + activation run on 8 NeuronCores,
data-parallel over batch (1 image per core).

Device strategy per core (v3):
  - x is pre-padded/transposed on the host to [cin=128, 258, 258] bf16 (zero
    SAME-padding baked in), so every DMA is a plain linear per-partition copy.
  - 3x3 conv = 9 accumulating matmuls per PSUM group: lhsT = w[cin,cout] per
    tap, rhs = shifted x rows ([2 rows x 256 cols] = 512 spatial AP), PSUM
    [cout=128, 512] fp32. Mid-stream the MM gap is the N=512 streaming floor
    (~216 ns), so the remaining time lives in the head and tail:
  - sqrt(2) is folded into the weights, so the epilogue is a single ScalarE
    op per group: y = Prelu(psum [+ sqrt2*bias], alpha=0.2), written directly
    as bf16 (host upcasts to fp32).
  - The image's first and last rows run as 1-row groups whose out-of-image
    tap rows (zero padding) are skipped: 6 matmuls instead of 9.
  - Head: slab 0's x DMA is split into 4 chunks (6/10/10/14 rows), and the
    first chunk is further split by partition halves across the two HWDGE
    queues (Sync + Scalar) so the first matmuls only wait ~3us of parallel
    DMA after queue spin-up (the GpSimd SWDGE queue spins up too slowly for
    head-critical data); ~42 dummy warm-up matmuls on a memset tile keep
    TensorE busy through the DMA spin-up so HAM is at K=8/8 (2.4 GHz) when
    the real matmuls start, with no idle gap long enough to let the HAM MID
    window re-throttle the PE.
  - Tail: output is DMA'd per 2-row group (128KB bf16) on the GpSimd DMA
    queue (decoupled from input prefetch); the last two groups go out on the
    by-then-idle Sync queue so the GpSimd queue drain doesn't serialize
    behind the final transfer.
"""

import math
from contextlib import ExitStack

import ml_dtypes
import numpy as np

import concourse.bacc as bacc
import concourse.bass as bass
import concourse.mybir as mybir
import concourse.tile as tile
from concourse.bass_utils import run_bass_kernel_spmd

B, H, W, CIN, COUT, KK, SDIM = 8, 256, 256, 128, 128, 3, 512
HP, WP = H + 2, W + 2  # zero-padded spatial dims (SAME padding for 3x3)
N_CORES = 8
ROWS_PER_SLAB = 32          # output rows per input slab
SLABS = H // ROWS_PER_SLAB  # 8
GROUP_ROWS = 2              # output rows per PSUM group (2*256 = 512 = 1 bank)
GROUPS_PER_SLAB = ROWS_PER_SLAB // GROUP_ROWS  # 16
N_WARMUP_MM = 54            # dummy matmuls covering DMA spin-up; sized to end
                            # at ~p90 of head-data arrival: ending early risks
                            # a HAM re-throttle (~1.8us), overshoot costs only
                            # the difference at ~56ns granularity
N_SYNCQ_TAIL_GROUPS = 2     # last groups whose output DMA rides the Sync queue
N_TAIL_SINGLE_ROWS = 2      # last output rows computed as 1-row groups (short tail)

# slab-0 chunks: (first input row, rows, first group, groups). Group g reads
# input rows 2g..2g+3; chunks overlap by 2 rows so every group is whole.
S0_CHUNKS = [(0, 6, 0, 2), (4, 10, 2, 4), (12, 10, 6, 4), (20, 14, 10, 6)]

BF16 = mybir.dt.bfloat16
F32 = mybir.dt.float32
SQRT2 = float(np.sqrt(np.float32(2.0)))


def _effective_weight(style, kernel, w_mod, b_mod):
    """Exactly the reference weight math, in fp32 numpy."""
    style = np.asarray(style, np.float32)
    kernel = np.asarray(kernel, np.float32)
    w_mod = np.asarray(w_mod, np.float32)
    b_mod = np.asarray(b_mod, np.float32)

    he_std = np.float32(1.0) / np.sqrt(np.float32(KK * KK * CIN))
    w = kernel * he_std
    s = (style @ w_mod + b_mod + np.float32(1.0)).astype(np.float32)
    s = s * (np.float32(1.0) / np.max(np.abs(s)))
    w = w * s[0][None, None, :, None]
    d = np.float32(1.0) / np.sqrt(
        np.sum(np.square(w), axis=(0, 1, 2), dtype=np.float32) + np.float32(1e-8)
    )
    w = w * d[None, None, None, :]
    return w.astype(np.float32)  # [3, 3, cin, cout]


def _build_program(with_noise: bool, with_bias: bool):
    # Bacc (not raw Bass): its compile() splits multi-sem sync waits into
    # event semaphores — TRN2 allows at most one wait per instruction.
    nc = bacc.Bacc(trn_type="TRN2")
    x = nc.declare_dram_parameter("x", [CIN, HP * WP], BF16, isOutput=False)
    w = nc.declare_dram_parameter("w", [CIN, 9 * COUT], BF16, isOutput=False)
    if with_bias:
        ab = nc.declare_dram_parameter("ab", [COUT, 1], F32, isOutput=False)
    if with_noise:
        nz = nc.declare_dram_parameter("nz", [1, H * W], BF16, isOutput=False)
        ones = nc.declare_dram_parameter("ones", [1, COUT], BF16, isOutput=False)
    y = nc.declare_dram_parameter("y", [COUT, H * W], BF16, isOutput=True)

    slab_rows_in = ROWS_PER_SLAB + 2  # input halo rows per slab

    with ExitStack() as ctx:
        tc = ctx.enter_context(tile.TileContext(nc))
        consts = ctx.enter_context(tc.tile_pool(name="consts", bufs=1))
        s0pool = ctx.enter_context(tc.tile_pool(name="s0", bufs=1))
        xpool = ctx.enter_context(tc.tile_pool(name="x", bufs=3))
        # Output staging: enough buffers to ride out the head transient while
        # the output DMA queue spins up.
        opool = ctx.enter_context(tc.tile_pool(name="out", bufs=10))
        pspool = ctx.enter_context(tc.tile_pool(name="ps", bufs=7, space="PSUM"))
        wupool = ctx.enter_context(tc.tile_pool(name="wups", bufs=1, space="PSUM"))
        if with_noise:
            nzpool = ctx.enter_context(tc.tile_pool(name="nz", bufs=2))

        # PE warm-up: dummy matmuls on a memset tile into a scratch PSUM bank
        # (never read). They run while the DMA queues spin up and the first
        # x chunk streams in, flipping HAM to K=8/8 before the real matmuls.
        # (Tile requires the dummy to be written, so a cheap GpSimd memset.)
        dummy = consts.tile([CIN, COUT], BF16)
        nc.gpsimd.memset(dummy[:], 0.0)
        wps = wupool.tile([COUT, COUT], F32)
        for _ in range(N_WARMUP_MM):
            nc.tensor.matmul(wps[:], dummy[:], dummy[:], start=True, stop=True)

        # Weights (and bias) go first on the Sync HWDGE queue: the GpSimd
        # queue is SWDGE with a multi-us spin-up, so weights parked there
        # arrive after the first x chunk and stall the first matmuls.
        wt = consts.tile([CIN, 9 * COUT], BF16)
        nc.sync.dma_start(wt[:], w[:])
        if with_bias:
            abt = consts.tile([COUT, 1], F32)
            nc.sync.dma_start(abt[:], ab[:])
        if with_noise:
            onest = consts.tile([1, COUT], BF16)
            nc.sync.dma_start(onest[:], ones[:])

        # Slab 0 arrives in small chunks so group 0 starts ~3us earlier. The
        # first chunk is further split by partition halves across the two
        # HWDGE queues (Sync + Scalar) so the head-critical bytes land in
        # parallel with the weight DMA.
        s0_views = []  # (first_group, n_groups, view, first_input_row)
        for ci, (row0, nrows, g0, ngroups) in enumerate(S0_CHUNKS):
            ck = s0pool.tile([CIN, nrows * WP], BF16, name=f"s0c{row0}")
            src = x[:, row0 * WP : (row0 + nrows) * WP]
            if ci == 0:
                # 48/80 split: the Sync queue also carries the 0.3MB weight
                # DMA, the Scalar queue is a bit slower — this equalizes when
                # the two halves (and the weights) finish.
                nc.sync.dma_start(ck[0:48, :], src[0:48, :])
                nc.scalar.dma_start(ck[48:128, :], src[48:128, :])
            else:
                nc.sync.dma_start(ck[:], src)
            s0_views.append(
                (g0, ngroups, ck[:].rearrange("p (r c) -> p r c", c=WP), row0)
            )

        def slab0_view(g):
            for g0, ngroups, view, row0 in s0_views:
                if g0 <= g < g0 + ngroups:
                    return view, 2 * g - row0
            raise AssertionError(g)

        for slab in range(SLABS):
            r0 = slab * ROWS_PER_SLAB  # first output row of the slab
            if slab > 0:
                xt = xpool.tile([CIN, slab_rows_in * WP], BF16)
                nc.sync.dma_start(
                    xt[:], x[:, r0 * WP : (r0 + slab_rows_in) * WP]
                )
                xv = xt[:].rearrange("p (r c) -> p r c", c=WP)
            if with_noise:
                nzt = nzpool.tile([1, ROWS_PER_SLAB * W], BF16)
                nc.sync.dma_start(nzt[:], nz[:, r0 * W : (r0 + ROWS_PER_SLAB) * W])

            # (row-in-slab, n-rows, skip_dh) work items. The image's first and
            # last rows run as 1-row groups: their out-of-image tap rows only
            # multiply the zero padding, so those 3 matmuls are skipped
            # (skip_dh); the last rows' short groups also halve the final
            # ACT + output DMA after the last matmul.
            if slab == 0:
                items = [(0, 1, 0), (1, 1, None)]
                items += [
                    (2 + g * GROUP_ROWS, GROUP_ROWS, None)
                    for g in range((ROWS_PER_SLAB - 2) // GROUP_ROWS)
                ]
            elif slab == SLABS - 1 and N_TAIL_SINGLE_ROWS:
                items = [
                    (g * GROUP_ROWS, GROUP_ROWS, None)
                    for g in range(GROUPS_PER_SLAB - N_TAIL_SINGLE_ROWS // GROUP_ROWS)
                ]
                items += [
                    (ROWS_PER_SLAB - N_TAIL_SINGLE_ROWS + k, 1, None)
                    for k in range(N_TAIL_SINGLE_ROWS)
                ]
                items[-1] = (items[-1][0], 1, 2)  # image row 255: padded below
            else:
                items = [
                    (g * GROUP_ROWS, GROUP_ROWS, None) for g in range(GROUPS_PER_SLAB)
                ]

            for rr, nrows, skip_dh in items:
                if slab == 0:
                    gv, grr = slab0_view(rr // GROUP_ROWS)
                    grr += rr % GROUP_ROWS
                else:
                    gv, grr = xv, rr
                ps = pspool.tile([COUT, nrows * W], F32)
                taps = [t for t in range(9) if divmod(t, 3)[0] != skip_dh]
                for i, t in enumerate(taps):
                    dh, dw = divmod(t, 3)
                    rhs = gv[:, grr + dh : grr + dh + nrows, dw : dw + W]
                    nc.tensor.matmul(
                        ps[:],
                        wt[:, t * COUT : (t + 1) * COUT],
                        rhs,
                        start=(i == 0),
                        stop=(i == len(taps) - 1 and not with_noise),
                    )
                if with_noise:
                    nc.tensor.matmul(
                        ps[:],
                        onest[:],
                        nzt[:, rr * W : (rr + nrows) * W],
                        start=False,
                        stop=True,
                    )
                ot = opool.tile([COUT, nrows * W], BF16)
                # weights carry sqrt2: y = prelu(ps + sqrt2*b, 0.2)
                nc.scalar.activation(
                    ot[:],
                    ps[:],
                    mybir.ActivationFunctionType.Prelu,
                    bias=abt[:, 0:1] if with_bias else 0.0,
                    scale=1.0,
                    alpha=0.2,
                )
                row = r0 + rr
                # Outputs ride the GpSimd DMA queue (decoupled from the input
                # prefetch FIFO); the final groups go via the by-then-idle
                # Sync queue so the GpSimd drain doesn't wait on them, and the
                # very last one is issued by the Scalar engine itself — its
                # ACT just produced the data, so no cross-engine sem hop sits
                # on the final critical chain.
                last = (
                    slab == SLABS - 1
                    and rr + nrows > ROWS_PER_SLAB - N_SYNCQ_TAIL_GROUPS * GROUP_ROWS
                )
                final = slab == SLABS - 1 and rr + nrows == ROWS_PER_SLAB
                eng = nc.scalar if final else (nc.sync if last else nc.gpsimd)
                eng.dma_start(y[:, row * W : (row + nrows) * W], ot[:])
    nc.finalize()  # Bacc.compile(): reg alloc + split multi-sem waits (TRN2)
    return nc


def _run(inputs, trace=False, **spmd_kwargs):
    x = np.asarray(inputs["x"])
    noise_strength = float(np.asarray(inputs["noise_strength"]).reshape(-1)[0])
    bias = np.asarray(inputs["bias"], np.float32)

    w_eff = _effective_weight(
        inputs["style"], inputs["kernel"], inputs["w_mod"], inputs["b_mod"]
    )
    wscale = np.float32(SQRT2)
    # [3,3,cin,cout] -> [cin, tap*cout], tap-major free dim
    w_dev = np.ascontiguousarray(
        (w_eff * wscale).transpose(2, 0, 1, 3).reshape(CIN, 9 * COUT)
    ).astype(ml_dtypes.bfloat16)

    # Pad + NHWC->NCHW per image, cast bf16. Zero borders bake in SAME padding.
    x_pad = np.zeros((B, CIN, HP, WP), dtype=ml_dtypes.bfloat16)
    x_pad[:, :, 1 : H + 1, 1 : W + 1] = x.transpose(0, 3, 1, 2).astype(
        ml_dtypes.bfloat16
    )

    with_bias = bool(np.any(bias != 0.0))
    with_noise = noise_strength != 0.0
    ab = (bias * wscale).reshape(COUT, 1).astype(np.float32)

    in_maps = []
    for b in range(B):
        m = {
            "x": np.ascontiguousarray(x_pad[b].reshape(CIN, HP * WP)),
            "w": w_dev,
        }
        if with_bias:
            m["ab"] = ab
        if with_noise:
            nzb = np.asarray(inputs["noise"], np.float32)[b, :, :, 0] * np.float32(
                wscale * noise_strength / 2.0
            )
            m["nz"] = nzb.reshape(1, H * W).astype(ml_dtypes.bfloat16)
            m["ones"] = np.ones((1, COUT), dtype=ml_dtypes.bfloat16)
        in_maps.append(m)

    nc = _build_program(with_noise, with_bias)
    res = run_bass_kernel_spmd(
        nc, in_maps, list(range(N_CORES)), trace=trace, **spmd_kwargs
    )

    out = np.empty((B, H, W, COUT), dtype=np.float32)
    for b in range(B):
        out[b] = (
            res.results[b]["y"]
            .astype(np.float32)
            .reshape(COUT, H, W)
            .transpose(1, 2, 0)
        )
    return out, res


def kernel(**inputs):
    out, _ = _run(inputs)
    return out
